# revision 3
# baseline (speedup 1.0000x reference)
"""EnhancedMultiHeadAttention on 8 Trainium2 NeuronCores — v3.

Sharding: 8 cores = 2 batches x 4 head-groups (4 heads / 256 columns each).
Per core: Q/K projections in bf16 (host-cast inputs, halving their DMA; the
resulting score noise averages out in softmax), V path in fp32r for accuracy,
attention in the fully transposed layout (scores^T = K @ Q^T; softmax
denominators from an extra expb column in V'), fp32r AV, bf16 out-projection
inputs are fp32r ctx against fp32r wo, f16 output partials (halving output
DMA; host sums in f32 and adds bo).

The Scalar engine's exp stream (~1ns/elem, 131us) is the hard floor; the
emission schedule hides everything else under it: only K block 0 + Q block 0
run before attention starts (with PE p-state warm-up matmuls at t=0); K
blocks 1-3, all of V-proj, and the remaining Q blocks stream in as paced
background chunks under the exp stream, with just-in-time input DMA.
"""

import sys

for _p in ("/opt/trn_rl_repo", "/root/.axon_site/_ro/trn_rl_repo"):
    if _p not in sys.path:
        sys.path.append(_p)

import numpy as np
import ml_dtypes

import concourse.bass as bass
import concourse.mybir as mybir
import concourse.tile as tile
from concourse import bacc
from concourse.bass_utils import run_bass_kernel_spmd
from concourse.masks import make_identity

F32 = mybir.dt.float32
F32R = mybir.dt.float32r
F16 = mybir.dt.float16
BF16 = mybir.dt.bfloat16

B, S, D = 2, 2048, 1024
H, DEPTH = 16, 64
NCORES = 8
GROUPS = 4                  # head-groups per batch
HC = H // GROUPS            # heads per core = 4
C = HC * DEPTH              # columns per core = 256
NPAIR = HC // 2             # head pairs per core = 2
DT = D // 128               # 8 d-tiles
TT = S // 128               # 16 k tiles
TP = TT // 2                # 8 k-tile pairs (position granularity)
QB = S // 512               # 4 q blocks
SCALE = 0.125               # 1/sqrt(DEPTH)


def build_nc(with_bias=False):
    nc = bacc.Bacc(None, target_bir_lowering=False)

    xq = nc.dram_tensor("xq", [D, S], F16, kind="ExternalInput")
    xk = nc.dram_tensor("xk", [D, S], F16, kind="ExternalInput")
    xv = nc.dram_tensor("xv", [D, S], F32R, kind="ExternalInput")
    wq = nc.dram_tensor("wq", [D, C], F16, kind="ExternalInput")
    wk = nc.dram_tensor("wk", [D, C], F16, kind="ExternalInput")
    wv = nc.dram_tensor("wv", [D, C], F32R, kind="ExternalInput")
    wo = nc.dram_tensor("wo", [C, D], F16, kind="ExternalInput")
    expb = nc.dram_tensor("expb", [S], F32, kind="ExternalInput")
    if with_bias:
        bq = nc.dram_tensor("bq", [C], F32, kind="ExternalInput")
        bk = nc.dram_tensor("bk", [C], F32, kind="ExternalInput")
        bv = nc.dram_tensor("bv", [C], F32, kind="ExternalInput")
    # f16 partials (summed to f32 on host) halve the output DMA traffic
    out = nc.dram_tensor("out", [S, D], F16, kind="ExternalOutput")

    with tile.TileContext(nc) as tc, nc.allow_low_precision(
        reason="bf16/f32r storage; all matmul accumulation stays fp32 in PSUM"
    ):
        with (
            tc.tile_pool(name="wpool", bufs=1) as wp,
            tc.tile_pool(name="qk", bufs=1) as qkp,
            tc.tile_pool(name="vsb", bufs=1) as vp,
            tc.tile_pool(name="ctxp", bufs=1) as cxp,
            tc.tile_pool(name="xs", bufs=20) as xsp,
            tc.tile_pool(name="vx", bufs=16) as vxp,
            tc.tile_pool(name="vT", bufs=2) as vTp,
            tc.tile_pool(name="pe", bufs=9) as pep,
            tc.tile_pool(name="nrm", bufs=3) as nrm,
            tc.tile_pool(name="sps", bufs=2, space="PSUM") as sps,
            tc.tile_pool(name="wps", bufs=4, space="PSUM") as wps,
            tc.tile_pool(name="dsc", bufs=4, space="DRAM") as dsc,
        ):
            # ---- PE p-state warm-up: ~4us of junk matmuls so the array is
            # at full clock when K-projection lands (ramp costs 2.7x early).
            junkw = wp.tile([128, 128], BF16)
            junkx = wp.tile([128, 512], BF16)
            nc.vector.memset(junkw[:], 0.0)
            nc.vector.memset(junkx[:], 0.0)
            for _ in range(8):
                jp = wps.tile([128, 512], F32, tag="w", name="jp")
                nc.tensor.matmul(jp[:], junkw[:], junkx[:])

            # ---- resident weights / constants (DMA issue order matters:
            # wk + xk block 0 first so K-projection starts ASAP) ----
            wk_sb = wp.tile([128, DT, C], F16)
            wq_sb = wp.tile([128, DT, C], F16)
            wv_sb = wp.tile([128, DT, C], F32R)
            wo_sb = wp.tile([128, 2, D], F16)
            expb_sb = wp.tile([128, TT], F32)
            ident = wp.tile([128, 128], F32)
            nc.sync.dma_start(wk_sb[:], wk.rearrange("(dt p) c -> p dt c", p=128))

            xk_r = xk.rearrange("(dt p) t -> dt p t", p=128)
            xq_r = xq.rearrange("(dt p) t -> dt p t", p=128)
            xv_r = xv.rearrange("(dt p) t -> dt p t", p=128)

            def dma_x_tb(x_r, tb, dt_=F16, pool=None):
                tiles = []
                for dt in range(DT):
                    xt = (pool or xsp).tile([128, 512], dt_, tag="xt",
                                            name="xt")
                    nc.sync.dma_start(xt[:], x_r[dt][:, tb * 512:(tb + 1) * 512])
                    tiles.append(xt)
                return tiles

            k_tiles = {0: dma_x_tb(xk_r, 0)}
            nc.sync.dma_start(wq_sb[:], wq.rearrange("(dt p) c -> p dt c", p=128))
            nc.sync.dma_start(expb_sb[:], expb.rearrange("(tt p) -> p tt", p=128))
            make_identity(nc, ident[:])
            q0_tiles = dma_x_tb(xq_r, 0)
            k_tiles[1] = dma_x_tb(xk_r, 1)
            if with_bias:
                bq_sb = wp.tile([128, 2], F32)
                bk_sb = wp.tile([128, 2], F32)
                bv_sb = wp.tile([128, 2], F32)
                nc.sync.dma_start(bk_sb[:], bk.rearrange("(ct p) -> p ct", p=128))
                nc.sync.dma_start(bq_sb[:], bq.rearrange("(ct p) -> p ct", p=128))
                nc.sync.dma_start(bv_sb[:], bv.rearrange("(ct p) -> p ct", p=128))

            # ---- persistent activations ----
            qT = [qkp.tile([128, S], F16, tag=f"qT{i}", name=f"qT{i}")
                  for i in range(NPAIR)]
            kT = [qkp.tile([128, S], F16, tag=f"kT{i}", name=f"kT{i}")
                  for i in range(NPAIR)]
            # V': per k-tile-pair [128, 2, HC, 65]; [:, s, h, :64] =
            # (V + bv)*expB for k-tile 2tp+s, [:, s, h, 64] = expB
            vs = [vp.tile([128, 2, HC, 65], F16, tag=f"vs{t}", name=f"vs{t}")
                  for t in range(TP)]
            ctx = [cxp.tile([128, S], F16, tag=f"ctx{i}", name=f"ctx{i}")
                   for i in range(NPAIR)]

            # ================= projection machinery =================
            def proj_ct(x_tiles, w_sb, ct):
                ps = wps.tile([128, 512], F32, tag="w", name="ps")
                for dt in range(DT):
                    nc.tensor.matmul(
                        ps[:],
                        w_sb[:, dt, ct * 128:(ct + 1) * 128],
                        x_tiles[dt][:],
                        start=(dt == 0),
                        stop=(dt == DT - 1),
                    )
                return ps

            def store_proj(dst, ct, tb, ps, b_sb):
                sl = dst[ct][:, tb * 512:(tb + 1) * 512]
                if with_bias:
                    nc.vector.tensor_scalar_add(sl, ps[:], b_sb[:, ct:ct + 1])
                else:
                    nc.vector.tensor_copy(sl, ps[:])

            def v_ct(x_tiles, ct, tb):
                """V-projection chunk: one (pair, 512-token block) -> vs."""
                ps = proj_ct(x_tiles, wv_sb, ct)
                vt = vTp.tile([128, 512], F32, tag="vt", name="vt")
                if with_bias:
                    nc.vector.tensor_scalar_add(vt[:], ps[:], bv_sb[:, ct:ct + 1])
                else:
                    nc.vector.tensor_copy(vt[:], ps[:])
                for j in range(4):
                    kt = tb * 4 + j
                    tp_, s = divmod(kt, 2)
                    tpp = wps.tile([128, 128], F32, tag="w", name="tpp")
                    nc.tensor.transpose(
                        tpp[:], vt[:, j * 128:(j + 1) * 128], ident[:])
                    for hh in range(2):
                        h = ct * 2 + hh
                        nc.vector.tensor_scalar_mul(
                            vs[tp_][:, s, h, 0:64],
                            tpp[:, hh * 64:(hh + 1) * 64],
                            expb_sb[:, kt:kt + 1],
                        )
                        nc.vector.tensor_copy(
                            vs[tp_][:, s, h, 64:65], expb_sb[:, kt:kt + 1])

            # ================= attention machinery =================
            oout = out.rearrange("(qt p) n -> qt p n", p=128)
            pending = {}   # (qb, pr, ktp) -> pe2 tile
            avs = {}       # (qb, pr) -> [av0, av1]

            def emit_qk_exp(qb, pr, ktp):
                qsl = slice(qb * 512, (qb + 1) * 512)
                pe2 = pep.tile([128, 2, 1024], F16, tag="pe", name="pe2")
                for s in range(2):
                    kt = 2 * ktp + s
                    ksl = slice(kt * 128, (kt + 1) * 128)
                    st = sps.tile([128, 1024], F32, tag="s", name="st")
                    for hh in range(2):
                        psl = slice(hh * 64, (hh + 1) * 64)
                        nc.tensor.matmul(
                            st[:, hh * 512:(hh + 1) * 512],
                            kT[pr][psl, ksl],
                            qT[pr][psl, qsl],
                        )
                    nc.scalar.activation(
                        pe2[:, s, :], st[:],
                        mybir.ActivationFunctionType.Exp, scale=SCALE)
                pending[qb, pr, ktp] = pe2

            def emit_av(qb, pr, ktp):
                pe2 = pending.pop((qb, pr, ktp))
                if ktp == 0:
                    avs[qb, pr] = [
                        wps.tile([65, 512], F32, tag="w", name="av")
                        for _ in range(2)
                    ]
                av = avs[qb, pr]
                for hh in range(2):
                    for s in range(2):
                        nc.tensor.matmul(
                            av[hh][:],
                            vs[ktp][:, s, pr * 2 + hh, :],
                            pe2[:, s, hh * 512:(hh + 1) * 512],
                            start=(ktp == 0 and s == 0),
                            stop=(ktp == TP - 1 and s == 1),
                        )
                if ktp == TP - 1:
                    emit_norm(qb, pr)

            def emit_norm(qb, pr):
                qsl = slice(qb * 512, (qb + 1) * 512)
                av = avs.pop((qb, pr))
                for hh in range(2):
                    rec = nrm.tile([1, 512], F32, tag="rec", name="rec")
                    nc.vector.reciprocal(rec[:], av[hh][64:65, :])
                    dr = dsc.tile([1, 512], F32, tag="dr", name="dr")
                    # norm-path DMAs ride the Activation DGE queue so they
                    # can't deadlock behind the bulk x/w input DMAs on sync
                    nc.scalar.dma_start(dr[:], rec[:])
                    dr_ap = dr[:]
                    bcast = bass.AP(
                        tensor=dr_ap.tensor, offset=dr_ap.offset,
                        ap=[[0, 64]] + [list(a) for a in dr_ap.ap[1:]],
                    )
                    bcs = nrm.tile([64, 512], F32, tag="bcs", name="bcs")
                    nc.scalar.dma_start(bcs[:], bcast)
                    if hh == 0:
                        nc.vector.tensor_mul(
                            ctx[pr][0:64, qsl], av[hh][0:64, :], bcs[:])
                    else:
                        tmp = nrm.tile([64, 512], F16, tag="tmp", name="tmp")
                        nc.vector.tensor_mul(tmp[:], av[hh][0:64, :], bcs[:])
                        nc.scalar.dma_start(ctx[pr][64:128, qsl], tmp[:])

            ots = {}

            def emit_outproj(qb, j):
                qt = qb * 4 + j // 2
                n = j % 2
                if n == 0:
                    ots[qt] = nrm.tile([128, D], F16, tag="ot", name="ot",
                                       bufs=2)
                ot = ots[qt]
                qts = slice(qt * 128, (qt + 1) * 128)
                po = wps.tile([128, 512], F32, tag="w", name="po")
                for ct in range(2):
                    nc.tensor.matmul(
                        po[:],
                        ctx[ct][:, qts],
                        wo_sb[:, ct, n * 512:(n + 1) * 512],
                        start=(ct == 0),
                        stop=(ct == 1),
                    )
                nc.vector.tensor_copy(ot[:, n * 512:(n + 1) * 512], po[:])
                if n == 1:
                    nc.sync.dma_start(oout[qt], ots.pop(qt)[:])

            # ================= emission schedule =================
            # Only K block 0 + Q block 0 up front; K blocks 1-3 are the
            # first background chunks (each position ktp of q-block 0 only
            # needs k-tiles 2*ktp..2*ktp+1, which arrive just ahead).
            for ct in range(2):
                ps = proj_ct(k_tiles[0], wk_sb, ct)
                store_proj(kT, ct, 0, ps, bk_sb if with_bias else None)
            for ct in range(2):
                ps = proj_ct(q0_tiles, wq_sb, ct)
                store_proj(qT, ct, 0, ps, bq_sb if with_bias else None)

            nc.sync.dma_start(wv_sb[:], wv.rearrange("(dt p) c -> p dt c", p=128))
            v_tiles = {}
            q_tiles = {}
            nc.sync.dma_start(wo_sb[:], wo.rearrange("(ct p) n -> p ct n", p=128))

            # Background chunks paced so the exp stream never starves.
            # K blocks 1-3 first (QK deadline: position p of q-block 0 needs
            # k-tiles 2p,2p+1), then V blocks (AV deadline: position
            # SKEW+ktp needs vs[ktp]; both head-pair chunks of a block run
            # adjacently so its x tiles retire quickly), then Q blocks.
            bg_sched = {
                0: [("k", 0, 1), ("k", 1, 1), ("kdma", 2)],
                1: [("k", 0, 2), ("k", 1, 2), ("kdma", 3)],
                2: [("k", 0, 3), ("k", 1, 3), ("vdma", 0)],
                4: [("v", 0, 0), ("vdma", 1)],
                5: [("v", 1, 0)],
                7: [("v", 0, 1), ("vdma", 2)],
                8: [("v", 1, 1)],
                10: [("v", 0, 2), ("vdma", 3)],
                11: [("v", 1, 2)],
                13: [("v", 0, 3)],
                9: [("qdma", 1)],
                12: [("q", 0, 1)],
                14: [("v", 1, 3), ("q", 1, 1)],
                26: [("qdma", 2)],
                29: [("q", 0, 2)],
                30: [("q", 1, 2)],
                42: [("qdma", 3)],
                45: [("q", 0, 3)],
                46: [("q", 1, 3)],
            }

            def run_bg(kind, a=None, b=None):
                if kind == "v":
                    v_ct(v_tiles[b], a, b)
                elif kind == "k":
                    ps = proj_ct(k_tiles[b], wk_sb, a)
                    store_proj(kT, a, b, ps, bk_sb if with_bias else None)
                elif kind == "vdma":
                    v_tiles[a] = dma_x_tb(xv_r, a, F32R, vxp)
                elif kind == "kdma":
                    k_tiles[a] = dma_x_tb(xk_r, a)
                elif kind == "qdma":
                    q_tiles[a] = dma_x_tb(xq_r, a)
                else:
                    ps = proj_ct(q_tiles[b], wq_sb, a)
                    store_proj(qT, a, b, ps, bq_sb if with_bias else None)

            flat = [(qb, pr, ktp)
                    for qb in range(QB) for pr in range(NPAIR)
                    for ktp in range(TP)]
            SKEW = 8
            OPDELAY = 3   # positions after a qb's last AV (ctx ready)
            op_sched = {}
            for qb in range(QB - 1):
                last_av_pos = (qb * NPAIR + NPAIR) * TP - 1 + SKEW
                for j in range(8):
                    op_sched[min(last_av_pos + OPDELAY + j,
                                 len(flat) - 1 - (7 - j))] = (qb, j)

            for i, pos in enumerate(flat):
                emit_qk_exp(*pos)
                for ev in bg_sched.get(i, ()):
                    run_bg(ev[0], *ev[1:])
                if i >= SKEW:
                    emit_av(*flat[i - SKEW])
                if i in op_sched:
                    emit_outproj(*op_sched[i])
            for i in range(len(flat) - SKEW, len(flat)):
                emit_av(*flat[i])
            for j in range(8):
                emit_outproj(QB - 1, j)

    nc.finalize()
    return nc


_NC = {}


def _get_nc(with_bias):
    if with_bias not in _NC:
        _NC[with_bias] = build_nc(with_bias)
    return _NC[with_bias]


def make_in_maps(query, key, value, temporal_bias, wq, wk, wv, wo,
                 bq=None, bk=None, bv=None):
    f = np.float32
    hf = np.float16
    xt = {}
    for b in range(B):
        xt["q", b] = np.ascontiguousarray(np.asarray(query[b], f).T.astype(hf))
        xt["k", b] = np.ascontiguousarray(np.asarray(key[b], f).T.astype(hf))
        xt["v", b] = np.ascontiguousarray(np.asarray(value[b], f).T)
    expb = np.exp(np.asarray(temporal_bias, f))
    wq = np.asarray(wq, f).astype(hf)
    wk = np.asarray(wk, f).astype(hf)
    wv = np.asarray(wv, f)
    wo = np.asarray(wo, f).astype(hf)
    in_maps = []
    for core in range(NCORES):
        b, g = divmod(core, GROUPS)
        cs = slice(g * C, (g + 1) * C)
        m = {
            "xq": xt["q", b],
            "xk": xt["k", b],
            "xv": xt["v", b],
            "wq": np.ascontiguousarray(wq[:, cs]),
            "wk": np.ascontiguousarray(wk[:, cs]),
            "wv": np.ascontiguousarray(wv[:, cs]),
            "wo": np.ascontiguousarray(wo[cs, :]),
            "expb": np.ascontiguousarray(expb[b]),
        }
        if bq is not None:
            m["bq"] = np.ascontiguousarray(np.asarray(bq, f)[cs])
            m["bk"] = np.ascontiguousarray(np.asarray(bk, f)[cs])
            m["bv"] = np.ascontiguousarray(np.asarray(bv, f)[cs])
        in_maps.append(m)
    return in_maps


def gather(results, bo):
    bo = np.asarray(bo, np.float32)
    out = np.zeros((B, S, D), np.float32)
    for core in range(NCORES):
        b = core // GROUPS
        out[b] += np.asarray(results[core]["out"], np.float32)
    out += bo[None, None, :]
    return out


def kernel(query, key, value, temporal_bias, wq, bq, wk, bk, wv, bv, wo, bo,
           _trace=False):
    with_bias = bool(np.any(bq) or np.any(bk) or np.any(bv))
    nc = _get_nc(with_bias)
    if with_bias:
        in_maps = make_in_maps(query, key, value, temporal_bias,
                               wq, wk, wv, wo, bq, bk, bv)
    else:
        in_maps = make_in_maps(query, key, value, temporal_bias,
                               wq, wk, wv, wo)
    res = run_bass_kernel_spmd(nc, in_maps, list(range(NCORES)), trace=_trace)
    out = gather(res.results, bo)
    if _trace:
        return out, res
    return out
